# revision 65
# baseline (speedup 1.0000x reference)
# Trainium2 Bass kernel for nn_Block_SA (dense_cnn self-attention block).
#
# Per-sample computation (C=64 channels, 64x64 spatial, N=4096 positions):
#   v   = relu(bn1(conv1x1(x)))                      # V for attention
#   s   = (x^T x) / sqrt(C)                          # [N, N] scores, Q=K=x
#   p   = softmax(s, axis=-1)
#   a   = V p^T  (a[d,n] = sum_m p[n,m] V[d,m])
#   z   = relu(bn2(depthwise3x3(a)))
#   out = bn3(conv1x1(z)) + x
#
# Distribution: batch B=8, one sample per NeuronCore (data parallel, no
# collectives). BN params are folded into conv weights on the host.
#
# On-chip algorithm (per core):
#   - Scores are computed TRANSPOSED: sT[m, n] tiles via matmul(lhsT=x[:,mtile],
#     rhs=x[:,nchunk]) so softmax's sum over m becomes a matmul reduction.
#     The K=64 contraction uses only half the PE array, so score matmuls are
#     row-packed two-at-a-time with tile_position (x duplicated on partitions
#     64-127) for ~2x PE throughput.
#   - exp() without max subtraction (scores/8 are small; fp32 is safe).
#   - The denominator sum_m exp(sT[m,n]) is obtained for free by augmenting
#     V^T with a ones column (row 64 of the AV accumulator).
#   - AV accumulates over all 32 m-tiles into one PSUM bank; 1/den is
#     broadcast across partitions with a K=1 PE matmul against a ones row.
#   - Depthwise 3x3 runs on the PE as 9 accumulating diagonal-weight matmuls
#     over shifted views of the attention output (PSUM accumulates taps);
#     the vector engine only applies bias+relu.
#   - conv3 + bias via augmented ones row; residual add; DMA out.
#
# Scheduling: the whole kernel is one flat software pipeline. The scalar
# engine's exp stream (85 instructions, ~134 us) is the roofline; everything
# else is arranged so ACT never starves. Tile scheduling priority = emission
# order, and the scheduler pops the earliest-emitted READY instruction, so:
#   - score-pair matmuls are emitted back-to-back (adjacent priority), so
#     the row-packed pair always issues together (the second half is ready
#     exactly when the first is — nothing can split them);
#   - AV matmuls are emitted in 3-matmul bursts aligned to exp groups,
#     lagging the exp stream by 2 groups, so the PE never head-blocks on an
#     exp and PSUM-bank switches happen once per burst, not per matmul;
#   - each chunk's normalization tail (PSUM drain, 1/den, broadcast,
#     y-scale, depthwise, conv3, output DMA) is spread across the next
#     chunk's pair stream so the [1,512] DVE reciprocal never head-blocks
#     the PE queue (in the original schedule that stall crossed the 3.4us
#     HAM window every chunk and kept the PE at half clock);
#   - single-row depthwise work (chunk-boundary rows) runs on the DVE, not
#     the PE: nine 64-wide matmuls cost more in accumulation-group churn
#     than ten DVE multiply-adds.
#
# Matmuls use float32r (full-rate fp32 mode on TRN2's PE; fp32 proper is
# 4 cycles/row). f32r operands must be produced by rounding-capable engines
# (DVE/ACT writes), not plain DMA.

import numpy as np

_EPS = 1e-5
_C = 64
_CP1 = 65
_N = 4096
_CH = 512          # free-dim chunk (one PSUM bank of fp32)
_NCH = _N // _CH   # 8 chunks
_MT = 128          # m-tile (partition dim of transposed score tiles)
_NMT = _N // _MT   # 32 m-tiles per chunk
_W = 64            # image width
_GS = [3] * 10 + [2]          # m-tiles per exp group (3 PSUM banks)
_NG = len(_GS)                # 11 exp groups per chunk
_G0 = [3 * g for g in range(_NG)]
_NCONST = 138 + 9 * 64 + 9  # w1aug | w3aug | w2p | b2p | diags | -w2p

_STATE = {}


def _build_program(reps=1):
    import concourse.bacc as bacc
    import concourse.tile as tile
    from concourse import mybir

    F32 = mybir.dt.float32
    F32R = mybir.dt.float32r
    BF16 = mybir.dt.bfloat16
    U32 = mybir.dt.uint32
    I16 = mybir.dt.int16
    I32 = mybir.dt.int32
    AF = mybir.ActivationFunctionType
    ALU = mybir.AluOpType
    ONE_BITS = 0x3F800000

    nc = bacc.Bacc(None)

    xd = nc.dram_tensor("x", [_C, _N], F32, kind="ExternalInput")
    # packed weights -> one DMA: cols 0:64 w1aug, 64:128 w3aug,
    # 128:137 w2p (rows 0:64), 137 b2p, 138:714 diag(w2p[:,k]) k=0..8
    cd = nc.dram_tensor("consts", [_CP1, _NCONST], F32, kind="ExternalInput")
    outd = nc.dram_tensor("out", [_C, _N], F32, kind="ExternalOutput")

    with tile.TileContext(nc) as tc:
        with (
            tc.tile_pool(name="persist", bufs=1) as pp,
            tc.tile_pool(name="small", bufs=2) as sp,
            tc.tile_pool(name="pt_pool", bufs=4) as ptp,
            tc.tile_pool(name="ps_pool", bufs=2, space="PSUM") as psp,
            tc.tile_pool(name="po_pool", bufs=1, space="PSUM") as pop,
            tc.tile_pool(name="aux_pool", bufs=1, space="PSUM") as auxp,
        ):
            def emit_all():
                # ---- input staging. x is loaded twice (partitions 0:64 and
                # 64:128) so score matmuls can row-pack; column-sliced with a
                # small first slice so the first score pair (and with it the
                # exp stream) starts as early as possible. The two partition
                # halves ride different DMA queues (sync / gpsimd).
                XSL = [(0, 512), (512, 1536), (1536, 2560), (2560, _N)]
                cs = pp.tile([_CP1, _NCONST], F32, name="cs", tag="cs")
                nc.scalar.dma_start(cs[:], cd[:])
                # dummy exp: pulls the ~1.3us ACT_TABLE_LOAD to t=0 so it
                # overlaps input DMA instead of delaying the first real exp
                dummy = sp.tile([1, 2], F32, name="dummy", tag="dummy")
                nc.vector.memset(dummy[:], 0.0)
                nc.scalar.activation(dummy[:], dummy[:], AF.Exp)
                # x + ones row (fp32): VT matmul lhsT and the residual add
                xo = pp.tile([_CP1, _N], F32, name="xo", tag="xo")
                nc.scalar.dma_start(xo[0:_C, 0:512], xd[:, 0:512])
                nc.scalar.dma_start(xo[0:_C, 512:1536], xd[:, 512:1536])
                nc.scalar.dma_start(xo[0:_C, 1536:_N], xd[:, 1536:_N])

                xs2 = pp.tile([_MT, _N], F32, name="xs2", tag="xs2")
                xr2 = pp.tile([_MT, _N], F32R, name="xr2", tag="xr2")
                nc.sync.dma_start(xs2[0:_C, 0:512], xd[:, 0:512])
                nc.gpsimd.dma_start(xs2[_C:_MT, 0:512], xd[:, 0:512])
                nc.gpsimd.memset(xo[_C:_CP1, :], 1.0)  # gates conv1 matmuls
                for a, b in XSL[1:]:
                    nc.sync.dma_start(xs2[0:_C, a:b], xd[:, a:b])
                    nc.gpsimd.dma_start(xs2[_C:_MT, a:b], xd[:, a:b])
                # only the first two column slices are converted to f32r here;
                # the rest are woven into chunk 0's pair stream so the DVE's
                # startup queue stays short (the chunk-0 bit-trick exps and
                # V^T relus need the DVE early)
                for a, b in XSL[:2]:
                    nc.vector.tensor_copy(xr2[:, a:b], xs2[:, a:b])

                w1s = cs[:, 0:64]
                w2s = cs[0:_C, 128:137]
                b2s = cs[0:_C, 137:138]
                w2n = cs[0:_C, 714:723]

                # V^T blocks: per m-tile a [128, 65] block (col 64 = ones).
                # bf16: the AV matmuls run bf16 x bf16 (same PE rate as f32r)
                # so the bit-trick exp can feed them via an int16 bitcast.
                vt = pp.tile([_MT, _NMT * _CP1], BF16, name="vt", tag="vt")
                vt3 = vt.rearrange("p (t c) -> p t c", c=_CP1)
                nc.gpsimd.memset(vt3[:, :, _C:_CP1], 1.0)

                # f32r-rounded weight copies (needed from the first tail on)
                w3r = pp.tile([_CP1, _C], F32R, name="w3r", tag="w3r")
                nc.vector.tensor_copy(w3r[:], cs[:, 64:128])
                dgr = pp.tile([_C, 9 * _C], F32R, name="dgr", tag="dgr")
                nc.vector.tensor_copy(dgr[:], cs[0:_C, 138:138 + 9 * _C])
                ones_r = pp.tile([1, _C], F32R, name="ones_r", tag="ones_r")
                nc.vector.memset(ones_r[:].bitcast(U32), ONE_BITS)
                # ones row AT partition 64: lhsT for the 1/den broadcast
                # matmul, so the whole den pipeline stays on partition 64
                # (DVE ops are lane-locked)
                ones64 = pp.tile([_CP1, _C], F32R, name="ones64", tag="ones64")
                nc.vector.memset(ones64[_C:_CP1, :].bitcast(U32), ONE_BITS)

                # normalized attention output (f32r: feeds the PE depthwise).
                # One zeroed pad row of 64 on each side so flat row-spanning
                # shifted reads stay in bounds.
                yrp = pp.tile([_C, _N + 2 * _W], F32R, name="yrp", tag="yrp")
                nc.gpsimd.memset(yrp[:, 0:_W].bitcast(U32), 0)
                nc.gpsimd.memset(yrp[:, _W + _N : _N + 2 * _W].bitcast(U32), 0)
                yr = yrp[:, _W : _W + _N]
                # post-depthwise activations (+ones row) feeding conv3
                zr = pp.tile([_CP1, _N], F32R, name="zr", tag="zr")
                nc.gpsimd.memset(zr[_C:_CP1, :].bitcast(U32), ONE_BITS)
                zrv = zr[0:_C, :].rearrange("c (h w) -> c h w", w=_W)

                # ---- V^T production (chunk 0 only): emitted lazily ahead of
                # AV demand. relu on DVE so the scalar engine runs Exp only.
                _vt_emitted = [0]

                def emit_vt_groups(need_mtiles):
                    while _vt_emitted[0] * 4 < need_mtiles:
                        g = _vt_emitted[0]
                        vps = auxp.tile([_MT, 4 * _C], F32, name="vps", tag="aux")
                        for j in range(4):
                            m = 4 * g + j
                            nc.tensor.matmul(
                                vps[:, _C * j : _C * (j + 1)],
                                lhsT=xo[:, _MT * m : _MT * (m + 1)],
                                rhs=w1s,
                                start=True,
                                stop=True,
                            )
                        nc.vector.tensor_relu(
                            vt3[:, 4 * g : 4 * (g + 1), 0:_C],
                            vps[:].rearrange("p (t c) -> p t c", c=_C),
                        )
                        _vt_emitted[0] += 1

                # ---- depthwise 3x3 on the PE: 9 accumulating diagonal matmuls
                # over flat shifted views of yr (full-row spans; the dx=+-1 taps
                # wrap across row edges and get small DVE fix-ups that subtract
                # the wrong contributions), then bias+relu on DVE.
                yrp3 = yrp.rearrange("c (h w) -> c h w", w=_W)  # row i = y row i-1

                def dw_row_dve(h):
                    # single output row entirely on the DVE (avoids nine tiny
                    # PE matmuls and their accumulation-group churn). SAME
                    # padding: partial-width slices skip the padded column.
                    def yrow(i, lo=0, hi=_W):  # y row i (flat 2D slice)
                        return yrp[:, _W * (i + 1) + lo : _W * (i + 1) + hi]

                    tmp = sp.tile([_C, _W], F32, name="dwt", tag="dwt", bufs=2)
                    nc.vector.tensor_scalar(
                        tmp[:], yrow(h), w2s[:, 4:5], b2s,
                        op0=ALU.mult, op1=ALU.add,
                    )
                    for k in (0, 1, 2, 3, 5, 6, 7, 8):
                        dy, dx = k // 3 - 1, k % 3 - 1
                        if not (0 <= h + dy < _W):
                            continue
                        if dx == -1:
                            dst, src = tmp[:, 1:_W], yrow(h + dy, 0, _W - 1)
                        elif dx == 1:
                            dst, src = tmp[:, 0 : _W - 1], yrow(h + dy, 1, _W)
                        else:
                            dst, src = tmp[:], yrow(h + dy)
                        nc.vector.scalar_tensor_tensor(
                            dst, src, w2s[:, k : k + 1], dst,
                            op0=ALU.mult, op1=ALU.add,
                        )
                    nc.vector.tensor_scalar_max(
                        zr[0:_C, _W * h : _W * (h + 1)], tmp[:], 0.0
                    )

                def emit_dw(h0, h1):
                    nh = h1 - h0
                    dwp = auxp.tile([_C, nh * _W], F32, name="dwp", tag="aux")
                    dwp3 = dwp.rearrange("c (h w) -> c h w", w=_W)
                    mms = []      # (k, out_lo, out_hi, src_lo, src_hi)
                    fixups = []
                    lastrow = []
                    for k in [4, 0, 1, 2, 3, 5, 6, 7, 8]:
                        dy, dx = k // 3 - 1, k % 3 - 1
                        hh0, hh1 = max(h0, -dy), min(h1, _W - dy)
                        if hh1 <= hh0:
                            continue
                        # tap k=8's flat span would read one element of the NEXT
                        # chunk's y (row hh1+1, w=0): do its last row exactly
                        flat_hh1 = hh1 - 1 if k == 8 else hh1
                        nhh = flat_hh1 - hh0
                        if nhh > 0:
                            src = _W + (hh0 + dy) * _W + dx
                            mms.append((k, (hh0 - h0) * _W, (flat_hh1 - h0) * _W,
                                        src, src + nhh * _W))
                            if dx != 0:
                                fixups.append((k, dy, dx, hh0, nhh))
                        if k == 8:
                            # f32r matmuls need even free counts; this 63-wide
                            # row goes on the DVE instead (after the PE group)
                            lastrow.append(hh1 - 1)
                    for i, (k, o0, o1, s0, s1) in enumerate(mms):
                        nc.tensor.matmul(
                            dwp[:, o0:o1],
                            lhsT=dgr[:, _C * k : _C * (k + 1)],
                            rhs=yrp[:, s0:s1],
                            start=(i == 0),
                            stop=(i == len(mms) - 1),
                            skip_group_check=True,
                        )
                    # tap k=8's last row: out[h, 0:63] += w2[8]*y[h+1, 1:64)
                    for h in lastrow:
                        dst8 = dwp3[:, h - h0 : h - h0 + 1, 0 : _W - 1]
                        nc.vector.scalar_tensor_tensor(
                            dst8,
                            yrp3[:, h + 2 : h + 3, 1:_W],
                            w2s[:, 8:9],
                            dst8,
                            op0=ALU.mult,
                            op1=ALU.add,
                        )
                    # subtract the wrap-around contribution of the dx=+-1 taps:
                    # dx=-1 polluted w=0 (read prev flat row's w=63), dx=+1
                    # polluted w=63 (read next flat row's w=0)
                    for k, dy, dx, hh0, nhh in fixups:
                        if dx == -1:
                            dst = dwp3[:, hh0 - h0 : hh0 - h0 + nhh, 0:1]
                            bad = yrp3[:, hh0 + dy : hh0 + dy + nhh, _W - 1 : _W]
                        else:
                            dst = dwp3[:, hh0 - h0 : hh0 - h0 + nhh, _W - 1 : _W]
                            bad = yrp3[:, hh0 + dy + 2 : hh0 + dy + 2 + nhh, 0:1]
                        nc.vector.scalar_tensor_tensor(
                            dst, bad, w2n[:, k : k + 1], dst, op0=ALU.mult, op1=ALU.add
                        )
                    nc.vector.tensor_scalar(
                        zrv[:, h0:h1, :], dwp3[:], b2s, 0.0, op0=ALU.add, op1=ALU.max
                    )

                def emit_conv3(c):
                    # conv3 (+bias via ones row) + residual + store
                    pc = auxp.tile([_C, _CH], F32, name="pc", tag="aux")
                    nc.tensor.matmul(
                        pc[:],
                        lhsT=w3r[:],
                        rhs=zr[:, _CH * c : _CH * (c + 1)],
                        start=True,
                        stop=True,
                    )
                    outt = sp.tile([_C, _CH], F32, name="outt", tag="outt", bufs=2)
                    nc.vector.tensor_tensor(
                        outt[:], pc[:], xo[0:_C, _CH * c : _CH * (c + 1)], op=ALU.add
                    )
                    nc.sync.dma_start(outd[:, _CH * c : _CH * (c + 1)], outt[:])

                def emit_av_group(po, pts, g, ci):
                    # one exp group's AV matmuls, back-to-back (single PSUM
                    # bank switch per burst)
                    if ci == 0:
                        emit_vt_groups(min(_G0[g] + _GS[g], _NMT))
                    for j in range(_GS[g]):
                        m = _G0[g] + j
                        nc.tensor.matmul(
                            po[0:_CP1, :],
                            lhsT=vt[:, _CP1 * m : _CP1 * (m + 1)],
                            rhs=pts[g][:, _CH * j : _CH * (j + 1)],
                            start=(m == 0),
                            stop=(m == _NMT - 1),
                            skip_group_check=True,
                        )

                # ---- flat pipelined main loop over n-chunks ----
                tail_q = []
                for ci in range(_NCH):
                    ps_t = [None] * _NG
                    pt_t = [None] * _NG
                    po = pop.tile([_MT, _CH], F32, name="po", tag="po")
                    next_exp = [0]
                    next_avg = [0]

                    for p in range(16):
                        # score pair (m, m+1): emitted back-to-back so the
                        # scheduler keeps them adjacent and the two row-halves
                        # run concurrently on the PE.
                        for m in (2 * p, 2 * p + 1):
                            g = m // 3 if m < 30 else 10
                            if ps_t[g] is None:
                                ps_t[g] = psp.tile(
                                    [_MT, _CH * _GS[g]], F32, name="ps", tag="ps"
                                )
                            j = m - _G0[g]
                            half = m % 2
                            rows = slice(_C * half, _C * (half + 1))
                            nc.tensor.matmul(
                                ps_t[g][:, _CH * j : _CH * (j + 1)],
                                lhsT=xr2[rows, _MT * m : _MT * (m + 1)],
                                rhs=xr2[rows, _CH * ci : _CH * (ci + 1)],
                                start=True,
                                stop=True,
                                tile_position=(_C * half, 0),
                            )
                        # exp groups whose scores are fully emitted. The exp
                        # stream is split across two engines: most groups run
                        # on ACT's spline exp; the last few per chunk run on
                        # the DVE as a Schraudolph bit-trick exp (one
                        # multiply-add with int32 convert-on-write, then the
                        # int bits reinterpreted as float). Softmax
                        # normalization cancels most of the ~3% sawtooth
                        # error (measured 2e-3 on the attention output).
                        while (
                            next_exp[0] < _NG
                            and _G0[next_exp[0]] + _GS[next_exp[0]] <= 2 * p + 2
                        ):
                            g = next_exp[0]
                            if g not in (7, 8):
                                pt_t[g] = ptp.tile(
                                    [_MT, _CH * _GS[g]], BF16, name="pt", tag="pt"
                                )
                                with nc.allow_low_precision(
                                    reason="softmax weights; normalization "
                                    "cancels bf16 rounding"
                                ):
                                    nc.scalar.activation(
                                        pt_t[g][:], ps_t[g][:], AF.Exp,
                                        scale=0.125,
                                    )
                            else:
                                ti = ptp.tile(
                                    [_MT, _CH * _GS[g]], I16, name="pt", tag="pt"
                                )
                                with nc.allow_low_precision(
                                    reason="bit-trick exp; softmax-normalized"
                                ):
                                    nc.vector.tensor_scalar(
                                        ti[:],
                                        ps_t[g][:],
                                        0.125 * 128.0 / 0.6931471805599453,
                                        16248.58,  # 127<<7 - 486411/2^16
                                        op0=ALU.mult,
                                        op1=ALU.add,
                                    )
                                pt_t[g] = ti[:].bitcast(BF16)
                            next_exp[0] += 1
                        # chunk 0: deferred xr2 f32r conversions (the conv1
                        # V^T bursts emit lazily inside emit_av_group — a
                        # prefetch here gets hoisted by the scheduler in
                        # front of the early score pairs and starves the
                        # exp stream for ~10us)
                        if ci == 0:
                            if p == 3:
                                a, b = XSL[2]
                                nc.vector.tensor_copy(xr2[:, a:b], xs2[:, a:b])
                            if p == 6:
                                a, b = XSL[3]
                                nc.vector.tensor_copy(xr2[:, a:b], xs2[:, a:b])
                        # previous chunk's tail, one piece per pair
                        if tail_q:
                            f = tail_q.pop(0)
                            if f is not None:
                                f()
                        # AV bursts lag the exp stream by 2 groups so their
                        # exp is complete (or nearly so) when the PE reaches
                        # them in its compiled order
                        while next_avg[0] <= next_exp[0] - 3:
                            emit_av_group(po, pt_t, next_avg[0], ci)
                            next_avg[0] += 1

                    # tail of chunk ci, interleaved into chunk ci+1's pairs
                    def mk(ci, po, pts):
                        st = {}

                        def avg(g):
                            def f():
                                emit_av_group(po, pts, g, ci)
                            return f

                        def drain():
                            usb = sp.tile([_CP1, _CH], F32, name="usb",
                                          tag="usb", bufs=2)
                            nc.vector.tensor_copy(usb[:], po[0:_CP1, :])
                            # 1/den via bit-trick seed + one Newton step, all
                            # on partition 64 (lane-locked DVE). Replaces the
                            # RECIPROCAL instruction, which takes 3.3us on a
                            # single-lane [1,512] tile but is modeled as cheap
                            # by the Tile scheduler - that mismatch kept
                            # placing the bcast matmul too early in the PE's
                            # compiled order, stalling the PE ~1.6us per chunk
                            # and tripping the HAM clock re-throttle.
                            den = usb[_C:_CP1, :]
                            r0i = sp.tile([_CP1, _CH], I32, name="r0i",
                                          tag="r0i", bufs=2)
                            with nc.allow_low_precision(
                                reason="reciprocal bit-trick + Newton; "
                                "feeds softmax normalization"
                            ):
                                nc.vector.tensor_scalar(
                                    r0i[_C:_CP1, :], den.bitcast(I32), -1.0,
                                    float(0x7EF311C3),
                                    op0=ALU.mult, op1=ALU.add,
                                )
                                r0 = r0i[_C:_CP1, :].bitcast(F32)
                                dt_ = sp.tile([_CP1, _CH], F32, name="dent",
                                              tag="dent", bufs=2)
                                nc.vector.tensor_tensor(
                                    dt_[_C:_CP1, :], den, r0, op=ALU.mult
                                )
                                nc.vector.tensor_scalar(
                                    dt_[_C:_CP1, :], dt_[_C:_CP1, :], -1.0,
                                    2.0, op0=ALU.mult, op1=ALU.add,
                                )
                                invden = sp.tile([_CP1, _CH], F32R,
                                                 name="invden", tag="invden",
                                                 bufs=2)
                                nc.vector.tensor_tensor(
                                    invden[_C:_CP1, :], dt_[_C:_CP1, :], r0,
                                    op=ALU.mult,
                                )
                            st["usb"], st["invden"] = usb, invden

                        def norm():
                            bcp = auxp.tile([_C, _CH], F32, name="bcp",
                                            tag="aux")
                            nc.tensor.matmul(
                                bcp[:], lhsT=ones64[_C:_CP1, :],
                                rhs=st["invden"][_C:_CP1, :],
                                start=True, stop=True,
                                tile_position=(_C, 0),
                            )
                            nc.vector.tensor_tensor(
                                yr[:, _CH * ci : _CH * (ci + 1)],
                                st["usb"][0:_C, :], bcp[:], op=ALU.mult,
                            )

                        def prev_close():
                            # boundary row of chunk ci-1 (needed this chunk's
                            # y), then its conv3 + store. In the steady state
                            # the row runs on the DVE (PE is contended); for
                            # the last chunk it runs in the epilogue where
                            # the PE is idle and the DVE is the serial chain.
                            if ci >= 1:
                                dw_row_dve(8 * ci - 1)
                                emit_conv3(ci - 1)

                        def dw_rows():
                            emit_dw(8 * ci, 8 * ci + 7)

                        # None = idle slot: the [1,512] DVE reciprocal takes
                        # ~3.3us on one lane (the scheduler's cost model
                        # underestimates it), so the bcast matmul in norm()
                        # must sit several pairs later in the PE's compiled
                        # order or the PE head-blocks on it and the HAM
                        # re-throttles the clock.
                        return [avg(9), avg(10), drain] + [None] * 8 + [
                            norm, prev_close, dw_rows]

                    tail_q = mk(ci, po, pt_t)

                # epilogue: last chunk's tail + final row + last conv3
                for f in tail_q:
                    if f is not None:
                        f()
                dw_row_dve(_N // _W - 1)  # last row (no dy=+1 tap)
                emit_conv3(_NCH - 1)

            if reps == 1:
                emit_all()
            else:
                with tc.For_i(0, reps, 1):
                    emit_all()

    nc.finalize()
    return nc


def _get_nc():
    if "nc" not in _STATE:
        _STATE["nc"] = _build_program()
    return _STATE["nc"]


def _prep_inputs(x, w1, bn1_g, bn1_b, bn1_m, bn1_v,
                 w2, bn2_g, bn2_b, bn2_m, bn2_v,
                 w3, bn3_g, bn3_b, bn3_m, bn3_v):
    f32 = np.float32
    x = np.asarray(x, f32)
    inv1 = np.asarray(bn1_g, f32) / np.sqrt(np.asarray(bn1_v, f32) + _EPS)
    w1p = np.asarray(w1, f32)[:, :, 0, 0] * inv1[:, None]
    b1p = np.asarray(bn1_b, f32) - np.asarray(bn1_m, f32) * inv1
    w1aug = np.concatenate([w1p.T, b1p[None, :]], axis=0)

    inv2 = np.asarray(bn2_g, f32) / np.sqrt(np.asarray(bn2_v, f32) + _EPS)
    w2p = np.asarray(w2, f32)[:, 0].reshape(_C, 9) * inv2[:, None]
    b2p = (np.asarray(bn2_b, f32) - np.asarray(bn2_m, f32) * inv2)[:, None]

    inv3 = np.asarray(bn3_g, f32) / np.sqrt(np.asarray(bn3_v, f32) + _EPS)
    w3p = np.asarray(w3, f32)[:, :, 0, 0] * inv3[:, None]
    b3p = np.asarray(bn3_b, f32) - np.asarray(bn3_m, f32) * inv3
    w3aug = np.concatenate([w3p.T, b3p[None, :]], axis=0)

    consts = np.zeros((_CP1, _NCONST), f32)
    consts[:, 0:64] = w1aug
    consts[:, 64:128] = w3aug
    consts[0:_C, 128:137] = w2p
    consts[0:_C, 137:138] = b2p
    for k in range(9):
        consts[0:_C, 138 + _C * k : 138 + _C * (k + 1)] = np.diag(w2p[:, k])
    consts[0:_C, 714:723] = -w2p

    B = x.shape[0]
    in_maps = []
    for i in range(B):
        in_maps.append({
            "x": np.ascontiguousarray(x[i].reshape(_C, _N)),
            "consts": consts,
        })
    return in_maps


def kernel(**inputs) -> np.ndarray:
    from concourse.bass_utils import run_bass_kernel_spmd

    in_maps = _prep_inputs(**inputs)
    nc = _get_nc()
    _STATE["in_maps"] = in_maps
    res = run_bass_kernel_spmd(nc, in_maps, list(range(len(in_maps))))
    out = np.stack(
        [r["out"].reshape(_C, _W, _W) for r in res.results]
    ).astype(np.float32)
    return out


def profile_exec_time():
    """Re-run the last inputs with NTFF tracing; returns exec time in ns."""
    from concourse.bass_utils import run_bass_kernel_spmd

    nc = _get_nc()
    in_maps = _STATE.get("in_maps")
    assert in_maps is not None, "call kernel() first"
    res = run_bass_kernel_spmd(nc, in_maps, list(range(len(in_maps))), trace=True)
    return res


# revision 68
# speedup vs baseline: 1.2706x; 1.2706x over previous
# Trainium2 Bass kernel for nn_Block_SA (dense_cnn self-attention block).
#
# Per-sample computation (C=64 channels, 64x64 spatial, N=4096 positions):
#   v   = relu(bn1(conv1x1(x)))                      # V for attention
#   s   = (x^T x) / sqrt(C)                          # [N, N] scores, Q=K=x
#   p   = softmax(s, axis=-1)
#   a   = V p^T  (a[d,n] = sum_m p[n,m] V[d,m])
#   z   = relu(bn2(depthwise3x3(a)))
#   out = bn3(conv1x1(z)) + x
#
# Distribution: batch B=8, one sample per NeuronCore (data parallel, no
# collectives). BN params are folded into conv weights on the host.
#
# On-chip algorithm (per core):
#   - Scores are computed TRANSPOSED: sT[m, n] tiles via matmul(lhsT=x[:,mtile],
#     rhs=x[:,nchunk]) so softmax's sum over m becomes a matmul reduction.
#     The K=64 contraction uses only half the PE array, so score matmuls are
#     row-packed two-at-a-time with tile_position (x duplicated on partitions
#     64-127) for ~2x PE throughput.
#   - exp() without max subtraction (scores/8 are small; fp32 is safe).
#   - The denominator sum_m exp(sT[m,n]) is obtained for free by augmenting
#     V^T with a ones column (row 64 of the AV accumulator).
#   - AV accumulates over all 32 m-tiles into one PSUM bank; 1/den is
#     broadcast across partitions with a K=1 PE matmul against a ones row.
#   - Depthwise 3x3 runs on the PE as 9 accumulating diagonal-weight matmuls
#     over shifted views of the attention output (PSUM accumulates taps);
#     the vector engine only applies bias+relu.
#   - conv3 + bias via augmented ones row; residual add; DMA out.
#
# Scheduling: the whole kernel is one flat software pipeline. The scalar
# engine's exp stream (85 instructions, ~134 us) is the roofline; everything
# else is arranged so ACT never starves. Tile scheduling priority = emission
# order, and the scheduler pops the earliest-emitted READY instruction, so:
#   - score-pair matmuls are emitted back-to-back (adjacent priority), so
#     the row-packed pair always issues together (the second half is ready
#     exactly when the first is — nothing can split them);
#   - AV matmuls are emitted in 3-matmul bursts aligned to exp groups,
#     lagging the exp stream by 2 groups, so the PE never head-blocks on an
#     exp and PSUM-bank switches happen once per burst, not per matmul;
#   - each chunk's normalization tail (PSUM drain, 1/den, broadcast,
#     y-scale, depthwise, conv3, output DMA) is spread across the next
#     chunk's pair stream so the [1,512] DVE reciprocal never head-blocks
#     the PE queue (in the original schedule that stall crossed the 3.4us
#     HAM window every chunk and kept the PE at half clock);
#   - single-row depthwise work (chunk-boundary rows) runs on the DVE, not
#     the PE: nine 64-wide matmuls cost more in accumulation-group churn
#     than ten DVE multiply-adds.
#
# Matmuls use float32r (full-rate fp32 mode on TRN2's PE; fp32 proper is
# 4 cycles/row). f32r operands must be produced by rounding-capable engines
# (DVE/ACT writes), not plain DMA.

import numpy as np

_EPS = 1e-5
_C = 64
_CP1 = 65
_N = 4096
_CH = 512          # free-dim chunk (one PSUM bank of fp32)
_NCH = _N // _CH   # 8 chunks
_MT = 128          # m-tile (partition dim of transposed score tiles)
_NMT = _N // _MT   # 32 m-tiles per chunk
_W = 64            # image width
_GS = [3] * 10 + [2]          # m-tiles per exp group (3 PSUM banks)
_NG = len(_GS)                # 11 exp groups per chunk
_G0 = [3 * g for g in range(_NG)]
_NCONST = 138 + 9 * 64 + 9  # w1aug | w3aug | w2p | b2p | diags | -w2p

_STATE = {}


def _build_program(reps=1):
    import concourse.bacc as bacc
    import concourse.tile as tile
    from concourse import mybir

    F32 = mybir.dt.float32
    F32R = mybir.dt.float32r
    BF16 = mybir.dt.bfloat16
    U32 = mybir.dt.uint32
    I16 = mybir.dt.int16
    I32 = mybir.dt.int32
    AF = mybir.ActivationFunctionType
    ALU = mybir.AluOpType
    ONE_BITS = 0x3F800000

    nc = bacc.Bacc(None)

    xd = nc.dram_tensor("x", [_C, _N], F32, kind="ExternalInput")
    # packed weights -> one DMA: cols 0:64 w1aug, 64:128 w3aug,
    # 128:137 w2p (rows 0:64), 137 b2p, 138:714 diag(w2p[:,k]) k=0..8
    cd = nc.dram_tensor("consts", [_CP1, _NCONST], F32, kind="ExternalInput")
    outd = nc.dram_tensor("out", [_C, _N], F32, kind="ExternalOutput")

    with tile.TileContext(nc) as tc:
        with (
            tc.tile_pool(name="persist", bufs=1) as pp,
            tc.tile_pool(name="small", bufs=2) as sp,
            tc.tile_pool(name="pt_pool", bufs=4) as ptp,
            tc.tile_pool(name="ps_pool", bufs=2, space="PSUM") as psp,
            tc.tile_pool(name="po_pool", bufs=1, space="PSUM") as pop,
            tc.tile_pool(name="aux_pool", bufs=1, space="PSUM") as auxp,
        ):
            def emit_all():
                # ---- input staging. x is loaded twice (partitions 0:64 and
                # 64:128) so score matmuls can row-pack; column-sliced with a
                # small first slice so the first score pair (and with it the
                # exp stream) starts as early as possible. The two partition
                # halves ride different DMA queues (sync / gpsimd).
                XSL = [(0, 512), (512, 1536), (1536, 2560), (2560, _N)]
                cs = pp.tile([_CP1, _NCONST], F32, name="cs", tag="cs")
                nc.scalar.dma_start(cs[:], cd[:])
                # dummy exp: pulls the ~1.3us ACT_TABLE_LOAD to t=0 so it
                # overlaps input DMA instead of delaying the first real exp
                dummy = sp.tile([1, 2], F32, name="dummy", tag="dummy")
                nc.vector.memset(dummy[:], 0.0)
                nc.scalar.activation(dummy[:], dummy[:], AF.Exp)
                # x + ones row (fp32): VT matmul lhsT and the residual add
                xo = pp.tile([_CP1, _N], F32, name="xo", tag="xo")
                nc.scalar.dma_start(xo[0:_C, 0:512], xd[:, 0:512])
                nc.scalar.dma_start(xo[0:_C, 512:1536], xd[:, 512:1536])
                nc.scalar.dma_start(xo[0:_C, 1536:_N], xd[:, 1536:_N])

                xs2 = pp.tile([_MT, _N], F32, name="xs2", tag="xs2")
                xr2 = pp.tile([_MT, _N], F32R, name="xr2", tag="xr2")
                nc.sync.dma_start(xs2[0:_C, 0:512], xd[:, 0:512])
                nc.gpsimd.dma_start(xs2[_C:_MT, 0:512], xd[:, 0:512])
                nc.gpsimd.memset(xo[_C:_CP1, :], 1.0)  # gates conv1 matmuls
                for a, b in XSL[1:]:
                    nc.sync.dma_start(xs2[0:_C, a:b], xd[:, a:b])
                    nc.gpsimd.dma_start(xs2[_C:_MT, a:b], xd[:, a:b])
                # only the first two column slices are converted to f32r here;
                # the rest are woven into chunk 0's pair stream so the DVE's
                # startup queue stays short (the chunk-0 bit-trick exps and
                # V^T relus need the DVE early)
                for a, b in XSL[:2]:
                    nc.vector.tensor_copy(xr2[:, a:b], xs2[:, a:b])

                w1s = cs[:, 0:64]
                # bf16 copies for the conv1 V^T matmuls: fp32 x fp32 matmul
                # streams at 1/4 rate on the PE; bf16 is full rate. Sliced so
                # the copies don't clog the DVE's startup queue.
                w1b = pp.tile([_CP1, _C], BF16, name="w1b", tag="w1b")
                nc.vector.tensor_copy(w1b[:], w1s)
                xob = pp.tile([_CP1, _N], BF16, name="xob", tag="xob")
                nc.vector.tensor_copy(xob[:, 0:512], xo[:, 0:512])
                w2s = cs[0:_C, 128:137]
                b2s = cs[0:_C, 137:138]
                w2n = cs[0:_C, 714:723]

                # V^T blocks: per m-tile a [128, 65] block (col 64 = ones).
                # bf16: the AV matmuls run bf16 x bf16 (same PE rate as f32r)
                # so the bit-trick exp can feed them via an int16 bitcast.
                vt = pp.tile([_MT, _NMT * _CP1], BF16, name="vt", tag="vt")
                vt3 = vt.rearrange("p (t c) -> p t c", c=_CP1)
                nc.gpsimd.memset(vt3[:, :, _C:_CP1], 1.0)

                # f32r-rounded weight copies (needed from the first tail on)
                w3r = pp.tile([_CP1, _C], F32R, name="w3r", tag="w3r")
                nc.vector.tensor_copy(w3r[:], cs[:, 64:128])
                dgr = pp.tile([_C, 9 * _C], F32R, name="dgr", tag="dgr")
                nc.vector.tensor_copy(dgr[:], cs[0:_C, 138:138 + 9 * _C])
                ones_r = pp.tile([1, _C], F32R, name="ones_r", tag="ones_r")
                nc.vector.memset(ones_r[:].bitcast(U32), ONE_BITS)
                # ones row AT partition 64: lhsT for the 1/den broadcast
                # matmul, so the whole den pipeline stays on partition 64
                # (DVE ops are lane-locked)
                ones64 = pp.tile([_CP1, _C], F32R, name="ones64", tag="ones64")
                nc.vector.memset(ones64[_C:_CP1, :].bitcast(U32), ONE_BITS)

                # normalized attention output (f32r: feeds the PE depthwise).
                # One zeroed pad row of 64 on each side so flat row-spanning
                # shifted reads stay in bounds.
                yrp = pp.tile([_C, _N + 2 * _W], F32R, name="yrp", tag="yrp")
                nc.gpsimd.memset(yrp[:, 0:_W].bitcast(U32), 0)
                nc.gpsimd.memset(yrp[:, _W + _N : _N + 2 * _W].bitcast(U32), 0)
                yr = yrp[:, _W : _W + _N]
                # post-depthwise activations (+ones row) feeding conv3
                zr = pp.tile([_CP1, _N], F32R, name="zr", tag="zr")
                nc.gpsimd.memset(zr[_C:_CP1, :].bitcast(U32), ONE_BITS)
                zrv = zr[0:_C, :].rearrange("c (h w) -> c h w", w=_W)

                # ---- V^T production (chunk 0 only): emitted lazily ahead of
                # AV demand. relu on DVE so the scalar engine runs Exp only.
                _vt_emitted = [0]

                def emit_vt_groups(need_mtiles):
                    while _vt_emitted[0] * 4 < need_mtiles:
                        g = _vt_emitted[0]
                        vps = auxp.tile([_MT, 4 * _C], F32, name="vps", tag="aux")
                        for j in range(4):
                            m = 4 * g + j
                            nc.tensor.matmul(
                                vps[:, _C * j : _C * (j + 1)],
                                lhsT=xob[:, _MT * m : _MT * (m + 1)],
                                rhs=w1b[:],
                                start=True,
                                stop=True,
                            )
                        nc.vector.tensor_relu(
                            vt3[:, 4 * g : 4 * (g + 1), 0:_C],
                            vps[:].rearrange("p (t c) -> p t c", c=_C),
                        )
                        _vt_emitted[0] += 1

                # ---- depthwise 3x3 on the PE: 9 accumulating diagonal matmuls
                # over flat shifted views of yr (full-row spans; the dx=+-1 taps
                # wrap across row edges and get small DVE fix-ups that subtract
                # the wrong contributions), then bias+relu on DVE.
                yrp3 = yrp.rearrange("c (h w) -> c h w", w=_W)  # row i = y row i-1

                def dw_row_dve(h):
                    # single output row entirely on the DVE (avoids nine tiny
                    # PE matmuls and their accumulation-group churn). SAME
                    # padding: partial-width slices skip the padded column.
                    def yrow(i, lo=0, hi=_W):  # y row i (flat 2D slice)
                        return yrp[:, _W * (i + 1) + lo : _W * (i + 1) + hi]

                    tmp = sp.tile([_C, _W], F32, name="dwt", tag="dwt", bufs=2)
                    nc.vector.tensor_scalar(
                        tmp[:], yrow(h), w2s[:, 4:5], b2s,
                        op0=ALU.mult, op1=ALU.add,
                    )
                    for k in (0, 1, 2, 3, 5, 6, 7, 8):
                        dy, dx = k // 3 - 1, k % 3 - 1
                        if not (0 <= h + dy < _W):
                            continue
                        if dx == -1:
                            dst, src = tmp[:, 1:_W], yrow(h + dy, 0, _W - 1)
                        elif dx == 1:
                            dst, src = tmp[:, 0 : _W - 1], yrow(h + dy, 1, _W)
                        else:
                            dst, src = tmp[:], yrow(h + dy)
                        nc.vector.scalar_tensor_tensor(
                            dst, src, w2s[:, k : k + 1], dst,
                            op0=ALU.mult, op1=ALU.add,
                        )
                    nc.vector.tensor_scalar_max(
                        zr[0:_C, _W * h : _W * (h + 1)], tmp[:], 0.0
                    )

                def emit_dw(h0, h1):
                    nh = h1 - h0
                    dwp = auxp.tile([_C, nh * _W], F32, name="dwp", tag="aux")
                    dwp3 = dwp.rearrange("c (h w) -> c h w", w=_W)
                    mms = []      # (k, out_lo, out_hi, src_lo, src_hi)
                    fixups = []
                    lastrow = []
                    for k in [4, 0, 1, 2, 3, 5, 6, 7, 8]:
                        dy, dx = k // 3 - 1, k % 3 - 1
                        hh0, hh1 = max(h0, -dy), min(h1, _W - dy)
                        if hh1 <= hh0:
                            continue
                        # tap k=8's flat span would read one element of the NEXT
                        # chunk's y (row hh1+1, w=0): do its last row exactly
                        flat_hh1 = hh1 - 1 if k == 8 else hh1
                        nhh = flat_hh1 - hh0
                        if nhh > 0:
                            src = _W + (hh0 + dy) * _W + dx
                            mms.append((k, (hh0 - h0) * _W, (flat_hh1 - h0) * _W,
                                        src, src + nhh * _W))
                            if dx != 0:
                                fixups.append((k, dy, dx, hh0, nhh))
                        if k == 8:
                            # f32r matmuls need even free counts; this 63-wide
                            # row goes on the DVE instead (after the PE group)
                            lastrow.append(hh1 - 1)
                    for i, (k, o0, o1, s0, s1) in enumerate(mms):
                        nc.tensor.matmul(
                            dwp[:, o0:o1],
                            lhsT=dgr[:, _C * k : _C * (k + 1)],
                            rhs=yrp[:, s0:s1],
                            start=(i == 0),
                            stop=(i == len(mms) - 1),
                            skip_group_check=True,
                        )
                    # tap k=8's last row: out[h, 0:63] += w2[8]*y[h+1, 1:64)
                    for h in lastrow:
                        dst8 = dwp3[:, h - h0 : h - h0 + 1, 0 : _W - 1]
                        nc.vector.scalar_tensor_tensor(
                            dst8,
                            yrp3[:, h + 2 : h + 3, 1:_W],
                            w2s[:, 8:9],
                            dst8,
                            op0=ALU.mult,
                            op1=ALU.add,
                        )
                    # subtract the wrap-around contribution of the dx=+-1 taps:
                    # dx=-1 polluted w=0 (read prev flat row's w=63), dx=+1
                    # polluted w=63 (read next flat row's w=0)
                    for k, dy, dx, hh0, nhh in fixups:
                        if dx == -1:
                            dst = dwp3[:, hh0 - h0 : hh0 - h0 + nhh, 0:1]
                            bad = yrp3[:, hh0 + dy : hh0 + dy + nhh, _W - 1 : _W]
                        else:
                            dst = dwp3[:, hh0 - h0 : hh0 - h0 + nhh, _W - 1 : _W]
                            bad = yrp3[:, hh0 + dy + 2 : hh0 + dy + 2 + nhh, 0:1]
                        nc.vector.scalar_tensor_tensor(
                            dst, bad, w2n[:, k : k + 1], dst, op0=ALU.mult, op1=ALU.add
                        )
                    nc.vector.tensor_scalar(
                        zrv[:, h0:h1, :], dwp3[:], b2s, 0.0, op0=ALU.add, op1=ALU.max
                    )

                def emit_conv3(c):
                    # conv3 (+bias via ones row) + residual + store
                    pc = auxp.tile([_C, _CH], F32, name="pc", tag="aux")
                    nc.tensor.matmul(
                        pc[:],
                        lhsT=w3r[:],
                        rhs=zr[:, _CH * c : _CH * (c + 1)],
                        start=True,
                        stop=True,
                    )
                    outt = sp.tile([_C, _CH], F32, name="outt", tag="outt", bufs=2)
                    nc.vector.tensor_tensor(
                        outt[:], pc[:], xo[0:_C, _CH * c : _CH * (c + 1)], op=ALU.add
                    )
                    nc.sync.dma_start(outd[:, _CH * c : _CH * (c + 1)], outt[:])

                def emit_av_group(po, pts, g, ci):
                    # one exp group's AV matmuls, back-to-back (single PSUM
                    # bank switch per burst)
                    if ci == 0:
                        emit_vt_groups(min(_G0[g] + _GS[g], _NMT))
                    for j in range(_GS[g]):
                        m = _G0[g] + j
                        nc.tensor.matmul(
                            po[0:_CP1, :],
                            lhsT=vt[:, _CP1 * m : _CP1 * (m + 1)],
                            rhs=pts[g][:, _CH * j : _CH * (j + 1)],
                            start=(m == 0),
                            stop=(m == _NMT - 1),
                            skip_group_check=True,
                        )

                # ---- flat pipelined main loop over n-chunks ----
                tail_q = []
                for ci in range(_NCH):
                    ps_t = [None] * _NG
                    pt_t = [None] * _NG
                    po = pop.tile([_MT, _CH], F32, name="po", tag="po")
                    next_exp = [0]
                    next_avg = [0]

                    for p in range(16):
                        # score pair (m, m+1): emitted back-to-back so the
                        # scheduler keeps them adjacent and the two row-halves
                        # run concurrently on the PE.
                        for m in (2 * p, 2 * p + 1):
                            g = m // 3 if m < 30 else 10
                            if ps_t[g] is None:
                                ps_t[g] = psp.tile(
                                    [_MT, _CH * _GS[g]], F32, name="ps", tag="ps"
                                )
                            j = m - _G0[g]
                            half = m % 2
                            rows = slice(_C * half, _C * (half + 1))
                            nc.tensor.matmul(
                                ps_t[g][:, _CH * j : _CH * (j + 1)],
                                lhsT=xr2[rows, _MT * m : _MT * (m + 1)],
                                rhs=xr2[rows, _CH * ci : _CH * (ci + 1)],
                                start=True,
                                stop=True,
                                tile_position=(_C * half, 0),
                            )
                        # exp groups whose scores are fully emitted. The exp
                        # stream is split across two engines: most groups run
                        # on ACT's spline exp; the last few per chunk run on
                        # the DVE as a Schraudolph bit-trick exp (one
                        # multiply-add with int32 convert-on-write, then the
                        # int bits reinterpreted as float). Softmax
                        # normalization cancels most of the ~3% sawtooth
                        # error (measured 2e-3 on the attention output).
                        while (
                            next_exp[0] < _NG
                            and _G0[next_exp[0]] + _GS[next_exp[0]] <= 2 * p + 2
                        ):
                            g = next_exp[0]
                            if g not in (7, 8):
                                pt_t[g] = ptp.tile(
                                    [_MT, _CH * _GS[g]], BF16, name="pt", tag="pt"
                                )
                                with nc.allow_low_precision(
                                    reason="softmax weights; normalization "
                                    "cancels bf16 rounding"
                                ):
                                    nc.scalar.activation(
                                        pt_t[g][:], ps_t[g][:], AF.Exp,
                                        scale=0.125,
                                    )
                            else:
                                ti = ptp.tile(
                                    [_MT, _CH * _GS[g]], I16, name="pt", tag="pt"
                                )
                                with nc.allow_low_precision(
                                    reason="bit-trick exp; softmax-normalized"
                                ):
                                    nc.vector.tensor_scalar(
                                        ti[:],
                                        ps_t[g][:],
                                        0.125 * 128.0 / 0.6931471805599453,
                                        16248.58,  # 127<<7 - 486411/2^16
                                        op0=ALU.mult,
                                        op1=ALU.add,
                                    )
                                pt_t[g] = ti[:].bitcast(BF16)
                            next_exp[0] += 1
                        # chunk 0: deferred xr2 f32r conversions (the conv1
                        # V^T bursts emit lazily inside emit_av_group — a
                        # prefetch here gets hoisted by the scheduler in
                        # front of the early score pairs and starves the
                        # exp stream for ~10us)
                        if ci == 0:
                            if p == 2:
                                nc.vector.tensor_copy(
                                    xob[:, 512:1536], xo[:, 512:1536]
                                )
                            if p == 3:
                                a, b = XSL[2]
                                nc.vector.tensor_copy(xr2[:, a:b], xs2[:, a:b])
                            if p == 5:
                                nc.vector.tensor_copy(
                                    xob[:, 1536:_N], xo[:, 1536:_N]
                                )
                            if p == 6:
                                a, b = XSL[3]
                                nc.vector.tensor_copy(xr2[:, a:b], xs2[:, a:b])
                        # previous chunk's tail, one piece per pair
                        if tail_q:
                            f = tail_q.pop(0)
                            if f is not None:
                                f()
                        # AV bursts lag the exp stream by 2 groups so their
                        # exp is complete (or nearly so) when the PE reaches
                        # them in its compiled order
                        while next_avg[0] <= next_exp[0] - 3:
                            emit_av_group(po, pt_t, next_avg[0], ci)
                            next_avg[0] += 1

                    # tail of chunk ci, interleaved into chunk ci+1's pairs
                    def mk(ci, po, pts):
                        st = {}

                        def avg(g):
                            def f():
                                emit_av_group(po, pts, g, ci)
                            return f

                        def drain():
                            usb = sp.tile([_CP1, _CH], F32, name="usb",
                                          tag="usb", bufs=2)
                            nc.vector.tensor_copy(usb[:], po[0:_CP1, :])
                            # 1/den via bit-trick seed + one Newton step, all
                            # on partition 64 (lane-locked DVE). Replaces the
                            # RECIPROCAL instruction, which takes 3.3us on a
                            # single-lane [1,512] tile but is modeled as cheap
                            # by the Tile scheduler - that mismatch kept
                            # placing the bcast matmul too early in the PE's
                            # compiled order, stalling the PE ~1.6us per chunk
                            # and tripping the HAM clock re-throttle.
                            den = usb[_C:_CP1, :]
                            r0i = sp.tile([_CP1, _CH], I32, name="r0i",
                                          tag="r0i", bufs=2)
                            with nc.allow_low_precision(
                                reason="reciprocal bit-trick + Newton; "
                                "feeds softmax normalization"
                            ):
                                nc.vector.tensor_scalar(
                                    r0i[_C:_CP1, :], den.bitcast(I32), -1.0,
                                    float(0x7EF311C3),
                                    op0=ALU.mult, op1=ALU.add,
                                )
                                r0 = r0i[_C:_CP1, :].bitcast(F32)
                                dt_ = sp.tile([_CP1, _CH], F32, name="dent",
                                              tag="dent", bufs=2)
                                nc.vector.tensor_tensor(
                                    dt_[_C:_CP1, :], den, r0, op=ALU.mult
                                )
                                nc.vector.tensor_scalar(
                                    dt_[_C:_CP1, :], dt_[_C:_CP1, :], -1.0,
                                    2.0, op0=ALU.mult, op1=ALU.add,
                                )
                                invden = sp.tile([_CP1, _CH], F32R,
                                                 name="invden", tag="invden",
                                                 bufs=2)
                                nc.vector.tensor_tensor(
                                    invden[_C:_CP1, :], dt_[_C:_CP1, :], r0,
                                    op=ALU.mult,
                                )
                            st["usb"], st["invden"] = usb, invden

                        def norm():
                            bcp = auxp.tile([_C, _CH], F32, name="bcp",
                                            tag="aux")
                            nc.tensor.matmul(
                                bcp[:], lhsT=ones64[_C:_CP1, :],
                                rhs=st["invden"][_C:_CP1, :],
                                start=True, stop=True,
                                tile_position=(_C, 0),
                            )
                            nc.vector.tensor_tensor(
                                yr[:, _CH * ci : _CH * (ci + 1)],
                                st["usb"][0:_C, :], bcp[:], op=ALU.mult,
                            )

                        def prev_close():
                            # boundary row of chunk ci-1 (needed this chunk's
                            # y), then its conv3 + store. In the steady state
                            # the row runs on the DVE (PE is contended); for
                            # the last chunk it runs in the epilogue where
                            # the PE is idle and the DVE is the serial chain.
                            if ci >= 1:
                                dw_row_dve(8 * ci - 1)
                                emit_conv3(ci - 1)

                        def dw_rows():
                            emit_dw(8 * ci, 8 * ci + 7)

                        # None = idle slot: the [1,512] DVE reciprocal takes
                        # ~3.3us on one lane (the scheduler's cost model
                        # underestimates it), so the bcast matmul in norm()
                        # must sit several pairs later in the PE's compiled
                        # order or the PE head-blocks on it and the HAM
                        # re-throttles the clock.
                        return [avg(9), avg(10), drain] + [None] * 8 + [
                            norm, prev_close, dw_rows]

                    tail_q = mk(ci, po, pt_t)

                # epilogue: last chunk's tail + final row + last conv3
                for f in tail_q:
                    if f is not None:
                        f()
                dw_row_dve(_N // _W - 1)  # last row (no dy=+1 tap)
                emit_conv3(_NCH - 1)

            if reps == 1:
                emit_all()
            else:
                with tc.For_i(0, reps, 1):
                    emit_all()

    nc.finalize()
    return nc


def _get_nc():
    if "nc" not in _STATE:
        _STATE["nc"] = _build_program()
    return _STATE["nc"]


def _prep_inputs(x, w1, bn1_g, bn1_b, bn1_m, bn1_v,
                 w2, bn2_g, bn2_b, bn2_m, bn2_v,
                 w3, bn3_g, bn3_b, bn3_m, bn3_v):
    f32 = np.float32
    x = np.asarray(x, f32)
    inv1 = np.asarray(bn1_g, f32) / np.sqrt(np.asarray(bn1_v, f32) + _EPS)
    w1p = np.asarray(w1, f32)[:, :, 0, 0] * inv1[:, None]
    b1p = np.asarray(bn1_b, f32) - np.asarray(bn1_m, f32) * inv1
    w1aug = np.concatenate([w1p.T, b1p[None, :]], axis=0)

    inv2 = np.asarray(bn2_g, f32) / np.sqrt(np.asarray(bn2_v, f32) + _EPS)
    w2p = np.asarray(w2, f32)[:, 0].reshape(_C, 9) * inv2[:, None]
    b2p = (np.asarray(bn2_b, f32) - np.asarray(bn2_m, f32) * inv2)[:, None]

    inv3 = np.asarray(bn3_g, f32) / np.sqrt(np.asarray(bn3_v, f32) + _EPS)
    w3p = np.asarray(w3, f32)[:, :, 0, 0] * inv3[:, None]
    b3p = np.asarray(bn3_b, f32) - np.asarray(bn3_m, f32) * inv3
    w3aug = np.concatenate([w3p.T, b3p[None, :]], axis=0)

    consts = np.zeros((_CP1, _NCONST), f32)
    consts[:, 0:64] = w1aug
    consts[:, 64:128] = w3aug
    consts[0:_C, 128:137] = w2p
    consts[0:_C, 137:138] = b2p
    for k in range(9):
        consts[0:_C, 138 + _C * k : 138 + _C * (k + 1)] = np.diag(w2p[:, k])
    consts[0:_C, 714:723] = -w2p

    B = x.shape[0]
    in_maps = []
    for i in range(B):
        in_maps.append({
            "x": np.ascontiguousarray(x[i].reshape(_C, _N)),
            "consts": consts,
        })
    return in_maps


def kernel(**inputs) -> np.ndarray:
    from concourse.bass_utils import run_bass_kernel_spmd

    in_maps = _prep_inputs(**inputs)
    nc = _get_nc()
    _STATE["in_maps"] = in_maps
    res = run_bass_kernel_spmd(nc, in_maps, list(range(len(in_maps))))
    out = np.stack(
        [r["out"].reshape(_C, _W, _W) for r in res.results]
    ).astype(np.float32)
    return out


def profile_exec_time():
    """Re-run the last inputs with NTFF tracing; returns exec time in ns."""
    from concourse.bass_utils import run_bass_kernel_spmd

    nc = _get_nc()
    in_maps = _STATE.get("in_maps")
    assert in_maps is not None, "call kernel() first"
    res = run_bass_kernel_spmd(nc, in_maps, list(range(len(in_maps))), trace=True)
    return res
